# revision 1
# baseline (speedup 1.0000x reference)
"""LIF neuron forward pass on 8 Trainium2 NeuronCores (raw Bass, SPMD).

Contract: kernel(x) with x: np.float32 [16, 32, 64, 64] returns the same
tuple as the reference: (mem_hist [50,16,32,64,64], spk_hist [50,...],
mem_f [16,32,64,64], encoded [16,32,64,64]), all float32.

Sharding: batch dim (16) split across 8 cores, 2 batches/core. Each core
runs the full 50-step time scan on its [128, 2048] shard; zero
communication. mem_hist/spk_hist stream out per step; `encoded` is
reconstructed host-side from spk_hist (bit-exact: spike counts are small
integers, and sum/50 in f32 matches the reference's jnp.sum/divide).

Per-step math (algebraic form of the reference's update; validated to be
within backend-to-backend numerical variation of the reference):
    memN = 0.9*mem + Y          Y = mask*xr2 - 7.5, xr2 = (x*0.1)*0.1
    spk  = (memN >= -55)
    mem' = 55*spk + memN
    h'   = (w_prev <= 0)*spk    } refractory bookkeeping:
    w'   = spk + h              }   mask = (w <= 0), w = spk + r2
    Y'   = (w' <= 0)*xr2 - 7.5
"""
import sys

for _p in ("/opt/trn_rl_repo", "/root/.axon_site/_ro/trn_rl_repo"):
    if _p not in sys.path:
        sys.path.append(_p)

import numpy as np
import concourse.bass as bass
import concourse.mybir as mybir

AT = mybir.AluOpType
AF = mybir.ActivationFunctionType
F32 = mybir.dt.float32

T = 50
P = 128
FREE = 2048          # 2*32*64*64 / 128
N_CORES = 8
INPUT_SHAPE = (16, 32, 64, 64)


def _build_lif(T=T, F=FREE, B=4):
    nc = bass.Bass()
    x = nc.dram_tensor("x", [P, F], F32, kind="ExternalInput")
    mem_hist = nc.dram_tensor("mem_hist", [T, P, F], F32, kind="ExternalOutput")
    spk_hist = nc.dram_tensor("spk_hist", [T, P, F], F32, kind="ExternalOutput")
    mem_f = nc.dram_tensor("mem_f", [P, F], F32, kind="ExternalOutput")

    with (
        nc.sbuf_tensor([P, F], F32) as xr2,
        nc.sbuf_tensor([P, 2, F], F32) as Y,
        nc.sbuf_tensor([P, 2, F], F32) as yq,
        nc.sbuf_tensor([P, 2, F], F32) as mem2,
        nc.sbuf_tensor([P, B, F], F32) as memN,
        nc.sbuf_tensor([P, B, F], F32) as spk,
        nc.sbuf_tensor([P, 2, F], F32) as w,
        nc.sbuf_tensor([P, 2, F], F32) as h,
        nc.semaphore() as s_in,
        nc.semaphore() as s_memN,
        nc.semaphore() as s_spk,
        nc.semaphore() as s_mem2,
        nc.semaphore() as s_yq,
        nc.semaphore() as s_Y,
        nc.semaphore() as s_dma_mem,
        nc.semaphore() as s_dma_spk,
        nc.Block() as block,
    ):
        @block.sync
        def _(sync):
            sync.dma_start(xr2[:, :], x[:, :]).then_inc(s_in, 16)
            for t in range(T):
                sync.wait_ge(s_memN, t + 1)
                sync.dma_start(mem_hist[t], memN[:, t % B, :]).then_inc(s_dma_mem, 16)
                sync.wait_ge(s_spk, t + 1)
                sync.dma_start(spk_hist[t], spk[:, t % B, :]).then_inc(s_dma_spk, 16)
            sync.wait_ge(s_mem2, T)
            sync.dma_start(mem_f[:, :], mem2[:, (T - 1) % 2, :]).then_inc(s_dma_mem, 16)

        @block.vector
        def _(vector):
            vector.wait_ge(s_in, 16)
            nc.vector.tensor_scalar(xr2[:, :], xr2[:, :], 0.1, 0.1, AT.mult, AT.mult)
            nc.vector.tensor_scalar(Y[:, 0, :], xr2[:, :], -7.5, None, AT.add)
            nc.vector.memset(mem2[:, 1, :], -75.0)
            nc.vector.memset(w[:, 1, :], 0.0)
            nc.vector.memset(h[:, 1, :], 0.0)
            for t in range(T):
                tb, t2, p2 = t % B, t % 2, (t - 1) % 2
                if t > 0:
                    vector.wait_ge(s_Y, t)
                if t >= B:
                    vector.wait_ge(s_dma_mem, 16 * (t - B + 1))
                nc.vector.scalar_tensor_tensor(
                    memN[:, tb, :], mem2[:, p2, :], 0.9, Y[:, t2, :], AT.mult, AT.add
                ).then_inc(s_memN, 1)
                if t >= B:
                    vector.wait_ge(s_dma_spk, 16 * (t - B + 1))
                nc.vector.tensor_scalar(
                    spk[:, tb, :], memN[:, tb, :], -55.0, None, AT.is_ge
                ).then_inc(s_spk, 1)
                nc.vector.scalar_tensor_tensor(
                    mem2[:, t2, :], spk[:, tb, :], 55.0, memN[:, tb, :], AT.mult, AT.add
                ).then_inc(s_mem2, 1)
                nc.vector.scalar_tensor_tensor(
                    h[:, t2, :], w[:, p2, :], 0.0, spk[:, tb, :], AT.is_le, AT.mult
                )
                nc.vector.tensor_tensor(
                    w[:, t2, :], spk[:, tb, :], h[:, p2, :], AT.add
                )
                if t < T - 1:
                    nc.vector.scalar_tensor_tensor(
                        yq[:, t2, :], w[:, t2, :], 0.0, xr2[:, :], AT.is_le, AT.mult
                    ).then_inc(s_yq, 1)

        @block.scalar
        def _(scalar):
            for t in range(T - 1):
                scalar.wait_ge(s_yq, t + 1)
                nc.scalar.activation(
                    Y[:, (t + 1) % 2, :], yq[:, t % 2, :], AF.Copy,
                    bias=-7.5, scale=1.0,
                ).then_inc(s_Y, 1)

    return nc


_NC_CACHE = {}


def _get_nc():
    if "nc" not in _NC_CACHE:
        _NC_CACHE["nc"] = _build_lif()
    return _NC_CACHE["nc"]


def kernel(x, _trace=False, _trace_kwargs=None):
    from concourse.bass_utils import run_bass_kernel_spmd

    x = np.asarray(x, dtype=np.float32)
    assert x.shape == INPUT_SHAPE, x.shape

    per_core = INPUT_SHAPE[0] // N_CORES  # 2 batches per core
    xs = x.reshape(N_CORES, per_core * 32 * 64 * 64)
    in_maps = [
        {"x": np.ascontiguousarray(xs[c].reshape(P, FREE))} for c in range(N_CORES)
    ]

    nc = _get_nc()
    kw = {}
    if _trace:
        kw = {"trace": True, "trace_kwargs": _trace_kwargs or {}}
    res = run_bass_kernel_spmd(nc, in_maps, core_ids=list(range(N_CORES)), **kw)

    mem_hist = np.empty((T,) + INPUT_SHAPE, np.float32)
    spk_hist = np.empty((T,) + INPUT_SHAPE, np.float32)
    mem_f = np.empty(INPUT_SHAPE, np.float32)
    sub = (per_core,) + INPUT_SHAPE[1:]
    for c in range(N_CORES):
        r = res.results[c]
        sl = slice(c * per_core, (c + 1) * per_core)
        mem_hist[:, sl] = r["mem_hist"].reshape((T,) + sub)
        spk_hist[:, sl] = r["spk_hist"].reshape((T,) + sub)
        mem_f[sl] = r["mem_f"].reshape(sub)

    encoded = spk_hist.sum(axis=0, dtype=np.float32) / np.float32(T)
    out = (mem_hist, spk_hist, mem_f, encoded)
    if _trace:
        return out, res
    return out


# revision 2
# speedup vs baseline: 1.1888x; 1.1888x over previous
"""LIF neuron forward pass on 8 Trainium2 NeuronCores (raw Bass, SPMD).

Contract: kernel(x) with x: np.float32 [16, 32, 64, 64] returns the same
tuple as the reference: (mem_hist [50,16,32,64,64], spk_hist [50,...],
mem_f [16,32,64,64], encoded [16,32,64,64]), all float32.

Sharding: batch dim (16) split across 8 cores, 2 batches/core. Each core
runs the full 50-step time scan on its [128, 2048] shard; zero
communication. mem_hist/spk_hist stream out per step; `encoded` is
reconstructed host-side from spk_hist (bit-exact: spike counts are small
integers, and sum/50 in f32 matches the reference's jnp.sum/divide).

Per-step math (algebraic form of the reference's update; validated to be
within backend-to-backend numerical variation of the reference):
    memN = 0.9*mem + Y          Y = mask*xr2 - 7.5, xr2 = (x*0.1)*0.1
    spk  = (memN >= -55)
    mem' = 55*spk + memN
    h'   = (w_prev <= 0)*spk    } refractory bookkeeping:
    w'   = spk + h              }   mask = (w <= 0), w = spk + r2
    Y'   = (w' <= 0)*xr2 - 7.5
"""
import sys

for _p in ("/opt/trn_rl_repo", "/root/.axon_site/_ro/trn_rl_repo"):
    if _p not in sys.path:
        sys.path.append(_p)

import numpy as np
import concourse.bass as bass
import concourse.mybir as mybir

AT = mybir.AluOpType
AF = mybir.ActivationFunctionType
F32 = mybir.dt.float32

T = 50
P = 128
FREE = 2048          # 2*32*64*64 / 128
N_CORES = 8
INPUT_SHAPE = (16, 32, 64, 64)


def _build_lif(T=T, F=FREE, B=4):
    nc = bass.Bass()
    x = nc.dram_tensor("x", [P, F], F32, kind="ExternalInput")
    mem_hist = nc.dram_tensor("mem_hist", [T, P, F], F32, kind="ExternalOutput")
    spk_hist = nc.dram_tensor("spk_hist", [T, P, F], F32, kind="ExternalOutput")
    mem_f = nc.dram_tensor("mem_f", [P, F], F32, kind="ExternalOutput")

    with (
        nc.sbuf_tensor([P, F], F32) as xr2,
        nc.sbuf_tensor([P, 2, F], F32) as Y,
        nc.sbuf_tensor([P, 2, F], F32) as yq,
        nc.sbuf_tensor([P, 2, F], F32) as mem2,
        nc.sbuf_tensor([P, B, F], F32) as memN,
        nc.sbuf_tensor([P, B, F], F32) as spk,
        nc.sbuf_tensor([P, 2, F], F32) as w,
        nc.sbuf_tensor([P, 2, F], F32) as h,
        nc.semaphore() as s_in,
        nc.semaphore() as s_memN,
        nc.semaphore() as s_spk,
        nc.semaphore() as s_mem2,
        nc.semaphore() as s_yq,
        nc.semaphore() as s_Y,
        nc.semaphore() as s_dma_mem,
        nc.semaphore() as s_dma_spk,
        nc.Block() as block,
    ):
        @block.sync
        def _(sync):
            sync.dma_start(xr2[:, :], x[:, :]).then_inc(s_in, 16)
            for t in range(T):
                sync.wait_ge(s_memN, t + 1)
                sync.dma_start(mem_hist[t], memN[:, t % B, :]).then_inc(s_dma_mem, 16)
                sync.wait_ge(s_spk, t + 1)
                sync.dma_start(spk_hist[t], spk[:, t % B, :]).then_inc(s_dma_spk, 16)
            sync.wait_ge(s_mem2, T)
            sync.dma_start(mem_f[:, :], mem2[:, (T - 1) % 2, :]).then_inc(s_dma_mem, 16)

        @block.vector
        def _(vector):
            vector.wait_ge(s_in, 16)
            nc.vector.tensor_scalar(xr2[:, :], xr2[:, :], 0.1, 0.1, AT.mult, AT.mult)
            nc.vector.tensor_scalar(Y[:, 0, :], xr2[:, :], -7.5, None, AT.add)
            nc.vector.memset(mem2[:, 1, :], -75.0)
            nc.vector.memset(w[:, 1, :], 0.0)
            nc.vector.memset(h[:, 1, :], 0.0)
            for t in range(T):
                tb, t2, p2 = t % B, t % 2, (t - 1) % 2
                if t > 0:
                    vector.wait_ge(s_Y, t)
                if t >= B:
                    vector.wait_ge(s_dma_mem, 16 * (t - B + 1))
                nc.vector.scalar_tensor_tensor(
                    memN[:, tb, :], mem2[:, p2, :], 0.9, Y[:, t2, :], AT.mult, AT.add
                ).then_inc(s_memN, 1)
                if t >= B:
                    vector.wait_ge(s_dma_spk, 16 * (t - B + 1))
                nc.vector.tensor_scalar(
                    spk[:, tb, :], memN[:, tb, :], -55.0, None, AT.is_ge
                ).then_inc(s_spk, 1)
                # w' and yq run early so ACT's Y' (critical path to the
                # next step's memN) overlaps with mem2/h below.
                nc.vector.tensor_tensor(
                    w[:, t2, :], spk[:, tb, :], h[:, p2, :], AT.add
                )
                if t < T - 1:
                    nc.vector.scalar_tensor_tensor(
                        yq[:, t2, :], w[:, t2, :], 0.0, xr2[:, :], AT.is_le, AT.mult
                    ).then_inc(s_yq, 1)
                nc.vector.scalar_tensor_tensor(
                    mem2[:, t2, :], spk[:, tb, :], 55.0, memN[:, tb, :], AT.mult, AT.add
                ).then_inc(s_mem2, 1)
                nc.vector.scalar_tensor_tensor(
                    h[:, t2, :], w[:, p2, :], 0.0, spk[:, tb, :], AT.is_le, AT.mult
                )

        @block.scalar
        def _(scalar):
            for t in range(T - 1):
                scalar.wait_ge(s_yq, t + 1)
                nc.scalar.activation(
                    Y[:, (t + 1) % 2, :], yq[:, t % 2, :], AF.Copy,
                    bias=-7.5, scale=1.0,
                ).then_inc(s_Y, 1)

    return nc


_NC_CACHE = {}


def _get_nc():
    if "nc" not in _NC_CACHE:
        _NC_CACHE["nc"] = _build_lif()
    return _NC_CACHE["nc"]


def kernel(x, _trace=False, _trace_kwargs=None):
    from concourse.bass_utils import run_bass_kernel_spmd

    x = np.asarray(x, dtype=np.float32)
    assert x.shape == INPUT_SHAPE, x.shape

    per_core = INPUT_SHAPE[0] // N_CORES  # 2 batches per core
    xs = x.reshape(N_CORES, per_core * 32 * 64 * 64)
    in_maps = [
        {"x": np.ascontiguousarray(xs[c].reshape(P, FREE))} for c in range(N_CORES)
    ]

    nc = _get_nc()
    kw = {}
    if _trace:
        kw = {"trace": True, "trace_kwargs": _trace_kwargs or {}}
    res = run_bass_kernel_spmd(nc, in_maps, core_ids=list(range(N_CORES)), **kw)

    mem_hist = np.empty((T,) + INPUT_SHAPE, np.float32)
    spk_hist = np.empty((T,) + INPUT_SHAPE, np.float32)
    mem_f = np.empty(INPUT_SHAPE, np.float32)
    sub = (per_core,) + INPUT_SHAPE[1:]
    for c in range(N_CORES):
        r = res.results[c]
        sl = slice(c * per_core, (c + 1) * per_core)
        mem_hist[:, sl] = r["mem_hist"].reshape((T,) + sub)
        spk_hist[:, sl] = r["spk_hist"].reshape((T,) + sub)
        mem_f[sl] = r["mem_f"].reshape(sub)

    encoded = spk_hist.sum(axis=0, dtype=np.float32) / np.float32(T)
    out = (mem_hist, spk_hist, mem_f, encoded)
    if _trace:
        return out, res
    return out
